# revision 10
# baseline (speedup 1.0000x reference)
"""Trainium2 Bass kernel for nn_ConvAttention.

The reference computes:
    fx = conv1x1(x, wf) + bf          # [B,1,H,W]
    gx = conv1x1(x, wg) + bg
    hx = conv1x1(x, wh) + bh
    a  = softmax(fx @ gx, axis=1)     # axis of size 1 -> identically 1.0
    o  = (hx @ a) * x                 # hx @ ones = row-sum broadcast over W

Because the softmax is over a size-1 axis it is exactly 1.0 everywhere, so
    o[b,c,i,j] = s[b,i] * x[b,c,i,j]
    s[b,i]     = sum_c sum_k x[b,c,i,k] * wh[c] + W * bh
wf/bf/wg/bg do not affect the output. The kernel streams x once through
SBUF - purely memory bound.

Sharding: pure data parallel over batch; 4 batches per core on 8 cores.
Weights (wh, bh) replicated.

v2 layout (vs the v1 all-PE contraction):
  * The channel contraction is factored as
        s[b,i] = sum_j wh_j . (sum_k x[b, c_j, i, k]) + W*bh
    i.e. a w-row-sum on the GpSimd (Pool) engine first - [128, hh*w] ->
    [128, hh] per c-chunk - then a tiny K=128, N=hh fp32 matmul per chunk
    on the PE. v1 streamed all of x through the PE in fp32 (2-pass HI/LO
    matmuls, 88 us/core of PE time, nearly the critical path); v2's PE
    work is ~2 us/core and the row-sums ride on the otherwise-idle Pool
    engine, leaving the DVE free for the output multiply.
  * The output is stored as fp16 (o = s*x quantized to half) and widened
    back to fp32 on the host. Max HW rel err ~5e-4 << the 2e-2 gate, and
    the HBM write traffic halves: 24 MiB/core round trip instead of 32.

Per-core pipeline: 8 groups of (batch, h-half); each group is 2 x 1 MiB
loads (c-chunk), 2 Pool row-sums, 2+1 tiny matmuls, bias add, 2 DVE
broadcast-multiplies into an fp16 tile, 1 x 1 MiB fp16 store.
"""

from contextlib import ExitStack

import numpy as np

B, C, H, W = 32, 256, 64, 64
N_CORES = 8
BS = B // N_CORES  # batches per core

_CACHE = {}


def _split_multi_waits(nc, mybir):
    """Walrus codegen allows only one sync-wait slot on most instruction
    encodings ("Too many sync wait commands"). Tile's sem assigner sometimes
    attaches 2-3. Hoist the extras onto standalone EventSemaphore
    instructions immediately before, on the same engine - semantically
    identical since engines execute their stream in order."""
    n = 0
    for f in nc.m.functions:
        for bb in f.blocks:
            new_insts = []
            for inst in bb.instructions:
                si = inst.sync_info
                ow = list(si.on_wait) if si and si.on_wait else []
                if len(ow) > 1:
                    for wv in ow[:-1]:
                        n += 1
                        evs = mybir.InstEventSemaphore(
                            name=f"evs_split_{n}",
                            ins=[],
                            outs=[],
                            engine=inst.engine,
                            bass_nofuse=True,
                            sync_info=mybir.SyncInfo(on_wait=[wv], on_update=[]),
                        )
                        nc.register_instruction(evs, overwrite=True)
                        new_insts.append(evs)
                    inst.sync_info = mybir.SyncInfo(
                        on_wait=[ow[-1]],
                        on_update=list(si.on_update) if si.on_update else [],
                    )
                new_insts.append(inst)
            bb.instructions = new_insts
    return n


def _build(bs, c, h, w):
    import concourse.bass as bass
    import concourse.tile as tile
    from concourse import mybir

    f32 = mybir.dt.float32
    f16 = mybir.dt.float16
    P = 128
    n_ch = c // P
    assert n_ch == 2 and c % P == 0
    # h-half groups: two 1 MiB loads each at full size
    n_half = 2 if h % 2 == 0 else 1
    hh = h // n_half
    fh = hh * w  # free elems per c-chunk within a group

    nc = bass.Bass("TRN2", target_bir_lowering=False, debug=False)
    x = nc.dram_tensor("x", [bs, c, h, w], f32, kind="ExternalInput").ap()
    wh = nc.dram_tensor("wh", [c], f32, kind="ExternalInput").ap()
    bh = nc.dram_tensor("bh", [1], f32, kind="ExternalInput").ap()
    o = nc.dram_tensor("o", [bs, c, h, w], f16, kind="ExternalOutput").ap()

    X = mybir.AxisListType.X

    with tile.TileContext(nc) as tc, ExitStack() as ctx:
        consts = ctx.enter_context(tc.tile_pool(name="consts", bufs=1))
        xpool = ctx.enter_context(tc.tile_pool(name="xp", bufs=bs))
        opool = ctx.enter_context(tc.tile_pool(name="op", bufs=4))
        rsp = ctx.enter_context(tc.tile_pool(name="rs", bufs=4))
        bcp = ctx.enter_context(tc.tile_pool(name="bc", bufs=4))
        pbp = ctx.enter_context(tc.tile_pool(name="pb", bufs=3, space="PSUM"))

        # wh as [128, n_ch]: column j holds wh[j*128:(j+1)*128]. Replicate
        # each column across 128 stationary columns (wh_bcast[p, j, m] =
        # wh[j*128+p]) so a single K=128 matmul per c-chunk computes the
        # channel contraction AND broadcasts s to all 128 partitions.
        wh_raw = consts.tile([P, n_ch], f32)
        nc.sync.dma_start(wh_raw[:], wh.rearrange("(j p) -> p j", p=P))
        wh_bcast = consts.tile([P, n_ch * P], f32)
        nc.vector.tensor_copy(
            wh_bcast[:].rearrange("p (j m) -> p j m", j=n_ch),
            wh_raw[:, :, None].broadcast_to((P, n_ch, P)),
        )
        # W*bh replicated on all partitions, for the final bias add
        bh_sb = consts.tile([P, 1], f32)
        nc.sync.dma_start(bh_sb[:], bh.to_broadcast((P, 1)))
        biasW = consts.tile([P, 1], f32)
        nc.scalar.mul(biasW[:], bh_sb[:], float(w))

        # Load plan: one SBUF tile per batch, filled by per-c-chunk DMAs
        # (2 MiB contiguous per chunk = best HBM efficiency). The LAST batch
        # is loaded in two pieces per chunk so its final piece is small.
        # Compute plan: per-batch groups of h-rows; the last batch tapers
        # (20/20/12/12) so the drain chain after the final load is short.
        if h == 64 and bs >= 2:
            load_rows = {b: [(0, h)] for b in range(bs - 1)}
            load_rows[bs - 1] = [(0, 40), (40, 24)]
            grp_rows = {b: [(0, hh), (hh, hh)] for b in range(bs - 1)}
            grp_rows[bs - 1] = [(0, 20), (20, 20), (40, 12), (52, 12)]
        else:
            load_rows = {b: [(0, h)] for b in range(bs)}
            grp_rows = {b: [(i * hh, hh) for i in range(n_half)] for b in range(bs)}

        groups = [(b, h0, hr) for b in range(bs) for (h0, hr) in grp_rows[b]]
        # Pool's tensor_tensor is ~1.7x slower per element than DVE's, and
        # DVE also carries all row-sums: ~3/8 of multiply work on DVE
        # balances (DVE ~57 us, Pool ~53 us). Alternate engines near the
        # tail so the last two groups' multiplies overlap.
        dve_mult = {g for g in range(len(groups)) if g % 2 == 1}

        xts = {}
        for b in range(bs):
            xt = xpool.tile([P, n_ch * h * w], f32)
            xts[b] = xt
            for ch in range(n_ch):
                for r0, rn in load_rows[b]:
                    nc.sync.dma_start(
                        xt[:, ch * h * w + r0 * w : ch * h * w + (r0 + rn) * w],
                        x[b, ch * P : (ch + 1) * P, r0 : r0 + rn].rearrange(
                            "c h w -> c (h w)"
                        ),
                    )

        for g, (b, h0, hr) in enumerate(groups):
            xt = xts[b]
            xg = xt[:].rearrange("c (j h w) -> c j h w", j=n_ch, h=h)[
                :, :, h0 : h0 + hr
            ]

            # Row-sums over w on the DVE: [128, (j hr) w] -> [128, (j hr)]
            rs = rsp.tile([P, n_ch * hr], f32)
            nc.vector.reduce_sum(
                rs[:].rearrange("c (j h) -> c j h", j=n_ch), xg, axis=X
            )

            # ps_b[m, i] = sum_j sum_p wh[j*128+p] * rs[p, j*hr+i] for all
            # m: contraction over channels and broadcast to 128 partitions
            # in one accumulating matmul pair.
            ps_b = pbp.tile([P, hr], f32)
            for ch in range(n_ch):
                nc.tensor.matmul(
                    ps_b[:],
                    lhsT=wh_bcast[:, ch * P : (ch + 1) * P],
                    rhs=rs[:, ch * hr : (ch + 1) * hr],
                    start=(ch == 0),
                    stop=(ch == n_ch - 1),
                )
            s128 = bcp.tile([P, hr], f32)
            nc.scalar.add(s128[:], ps_b[:], biasW[:])

            # o = s * x quantized to fp16, then store this group's rows
            ot = opool.tile([P, n_ch * hr * w], f16)
            eng = nc.vector if g in dve_mult else nc.gpsimd
            eng.tensor_mul(
                ot[:].rearrange("c (j h w) -> c j h w", j=n_ch, h=hr),
                xg,
                s128[:, None, :, None].broadcast_to((P, n_ch, hr, w)),
            )
            nc.scalar.dma_start(
                o[b, :, h0 : h0 + hr].rearrange("(j c) h w -> c j h w", c=P),
                ot[:].rearrange("c (j h w) -> c j h w", j=n_ch, h=hr),
            )
    _split_multi_waits(nc, mybir)
    return nc


def get_nc(bs=BS, c=C, h=H, w=W):
    key = (bs, c, h, w)
    if key not in _CACHE:
        _CACHE[key] = _build(bs, c, h, w)
    return _CACHE[key]


def kernel(x, wf, bf, wg, bg, wh, bh, **_unused):
    from concourse.bass_utils import run_bass_kernel_spmd

    x = np.ascontiguousarray(np.asarray(x, dtype=np.float32))
    wh = np.ascontiguousarray(np.asarray(wh, dtype=np.float32))
    bh = np.ascontiguousarray(np.asarray(bh, dtype=np.float32))

    in_maps = [
        {"x": x[k * BS : (k + 1) * BS], "wh": wh, "bh": bh} for k in range(N_CORES)
    ]
    # Tile scheduling is nondeterministic build-to-build and a rare schedule
    # can deadlock on hardware (NRT unrecoverable). Rebuilding produces a
    # fresh schedule, so retry with a clean build on any execution failure.
    last_err = None
    for attempt in range(3):
        try:
            nc = get_nc()
            res = run_bass_kernel_spmd(nc, in_maps, core_ids=list(range(N_CORES)))
            return np.concatenate(
                [
                    np.asarray(res.results[k]["o"], dtype=np.float32)
                    for k in range(N_CORES)
                ],
                axis=0,
            )
        except Exception as e:  # rebuild with a new schedule and retry
            last_err = e
            _CACHE.clear()
    raise last_err


# revision 11
# speedup vs baseline: 1.1279x; 1.1279x over previous
"""Trainium2 Bass kernel for nn_ConvAttention.

The reference computes:
    fx = conv1x1(x, wf) + bf          # [B,1,H,W]
    gx = conv1x1(x, wg) + bg
    hx = conv1x1(x, wh) + bh
    a  = softmax(fx @ gx, axis=1)     # axis of size 1 -> identically 1.0
    o  = (hx @ a) * x                 # hx @ ones = row-sum broadcast over W

Because the softmax is over a size-1 axis it is exactly 1.0 everywhere, so
    o[b,c,i,j] = s[b,i] * x[b,c,i,j]
    s[b,i]     = sum_c sum_k x[b,c,i,k] * wh[c] + W * bh
wf/bf/wg/bg do not affect the output. The kernel streams x once through
SBUF - purely memory bound. Sharding: pure data parallel over batch;
4 batches per core on 8 cores, weights replicated.

v3 pipeline (HW-measured evolution; see git-style history in test logs):
  * Loads are SWDGE (gpsimd) DMAs that cast fp32 -> bf16 in the DMA
    datapath (HW-measured: same rate as plain HWDGE loads). HBM read
    traffic is unchanged (16 MiB f32/core) but SBUF holds bf16, and the
    PE can then run 1-pass bf16 matmuls (fp32 matmuls are 2-pass HI/LO
    and were the v1 bottleneck at 88 us/core).
  * The channel contraction runs on the PE with a replicated stationary
    operand: wh_bcast[p, j*128+m] = wh[j*128+p] for all m, so
    psum[m, (i,k)] = sum_c wh[c] x[c,i,k] lands broadcast across all 128
    partitions. One accumulating matmul pair per 512-column block.
  * s = row-sum(psum) + W*bh: full-lane DVE reduce (PSUM -> SBUF) then a
    tiny ACT bias add. This replaces v2's 35 us of SBUF row-sum reduces
    with 17 us of PSUM reduces (the j-chunk dim is pre-contracted).
  * o = s * x on DVE (bf16 in, fp16 out; two mid groups on the Pool
    engine to shave DVE's peak). Output is stored as fp16 and widened to
    fp32 on the host: halves HBM write traffic (8 MiB/core), max rel err
    ~5e-3 << the 2e-2 gate.
  * The last batch is loaded and computed in tapering groups
    (24/24/8/8 rows) so the post-last-load drain chain is short.

DMA roofline for this traffic (25.2 MB/core, HW-measured with no
compute): 74.4 us end-to-end incl the ~7 us Tile preamble.
"""

from contextlib import ExitStack

import numpy as np

B, C, H, W = 32, 256, 64, 64
N_CORES = 8
BS = B // N_CORES  # batches per core

_CACHE = {}


def _split_multi_waits(nc, mybir):
    """Walrus codegen allows only one sync-wait slot on most instruction
    encodings ("Too many sync wait commands"). Tile's sem assigner sometimes
    attaches 2-3. Hoist the extras onto standalone EventSemaphore
    instructions immediately before, on the same engine - semantically
    identical since engines execute their stream in order."""
    n = 0
    for f in nc.m.functions:
        for bb in f.blocks:
            new_insts = []
            for inst in bb.instructions:
                si = inst.sync_info
                ow = list(si.on_wait) if si and si.on_wait else []
                if len(ow) > 1:
                    for wv in ow[:-1]:
                        n += 1
                        evs = mybir.InstEventSemaphore(
                            name=f"evs_split_{n}",
                            ins=[],
                            outs=[],
                            engine=inst.engine,
                            bass_nofuse=True,
                            sync_info=mybir.SyncInfo(on_wait=[wv], on_update=[]),
                        )
                        nc.register_instruction(evs, overwrite=True)
                        new_insts.append(evs)
                    inst.sync_info = mybir.SyncInfo(
                        on_wait=[ow[-1]],
                        on_update=list(si.on_update) if si.on_update else [],
                    )
                new_insts.append(inst)
            bb.instructions = new_insts
    return n


def _build(bs, c, h, w):
    import concourse.bass as bass
    import concourse.tile as tile
    from concourse import mybir

    f32 = mybir.dt.float32
    f16 = mybir.dt.float16
    bf16 = mybir.dt.bfloat16
    P = 128
    n_ch = c // P
    assert n_ch == 2 and c % P == 0
    hw = h * w

    nc = bass.Bass("TRN2", target_bir_lowering=False, debug=False)
    x = nc.dram_tensor("x", [bs, c, h, w], f32, kind="ExternalInput").ap()
    wh = nc.dram_tensor("wh", [c], f32, kind="ExternalInput").ap()
    bh = nc.dram_tensor("bh", [1], f32, kind="ExternalInput").ap()
    o = nc.dram_tensor("o", [bs, c, h, w], f16, kind="ExternalOutput").ap()

    X = mybir.AxisListType.X

    # Load pieces (per c-chunk) and compute groups (h-row ranges), with a
    # tapered tail on the last batch so the final drain chain is short.
    if h == 64 and bs >= 2:
        load_rows = {b: [(0, h)] for b in range(bs - 1)}
        load_rows[bs - 1] = [(0, 48), (48, 16)]
        grp_rows = {b: [(0, 32), (32, 32)] for b in range(bs - 1)}
        grp_rows[bs - 1] = [(0, 24), (24, 24), (48, 8), (56, 8)]
    else:
        hh = h // 2 if h % 2 == 0 else h
        load_rows = {b: [(0, h)] for b in range(bs)}
        grp_rows = {
            b: [(i * hh, hh) for i in range(h // hh)] for b in range(bs)
        }
    groups = [(b, h0, hr) for b in range(bs) for (h0, hr) in grp_rows[b]]
    # Multiply engine: DVE everywhere except two mid groups on Pool (Pool
    # is ~1.7x slower per element but otherwise idle after load issue).
    pool_mult = {g for g in range(len(groups)) if g in (4, 5)}

    with tile.TileContext(nc) as tc, ExitStack() as ctx:
        consts = ctx.enter_context(tc.tile_pool(name="consts", bufs=1))
        xpool = ctx.enter_context(tc.tile_pool(name="xp", bufs=bs))
        opool = ctx.enter_context(tc.tile_pool(name="op", bufs=4))
        spool = ctx.enter_context(tc.tile_pool(name="sp", bufs=4))
        rpool = ctx.enter_context(tc.tile_pool(name="rp", bufs=4))
        psp = ctx.enter_context(tc.tile_pool(name="ps", bufs=2, space="PSUM"))

        # wh as bf16 [128, n_ch*128]: wh_bcast[p, j*128+m] = wh[j*128+p]
        # for all m - the contraction matmul then broadcasts s to all
        # 128 output partitions for free.
        wh_raw = consts.tile([P, n_ch], f32)
        nc.sync.dma_start(wh_raw[:], wh.rearrange("(j p) -> p j", p=P))
        wh_bcast = consts.tile([P, n_ch * P], bf16)
        nc.vector.tensor_copy(
            wh_bcast[:].rearrange("p (j m) -> p j m", j=n_ch),
            wh_raw[:, :, None].broadcast_to((P, n_ch, P)),
        )
        # W*bh replicated on all partitions, for the final bias add
        bh_sb = consts.tile([P, 1], f32)
        nc.sync.dma_start(bh_sb[:], bh.to_broadcast((P, 1)))
        biasW = consts.tile([P, 1], f32)
        nc.scalar.mul(biasW[:], bh_sb[:], float(w))

        # All loads first: SWDGE fp32 -> bf16 cast DMAs on the Pool queue.
        # (Pool's multiply work is emitted after every load is issued.)
        xbs = {}
        for b in range(bs):
            xb = xpool.tile([P, n_ch * hw], bf16)
            xbs[b] = xb
            for ch in range(n_ch):
                for r0, rn in load_rows[b]:
                    nc.gpsimd.dma_start(
                        xb[:, ch * hw + r0 * w : ch * hw + (r0 + rn) * w],
                        x[b, ch * P : (ch + 1) * P, r0 : r0 + rn].rearrange(
                            "c h w -> c (h w)"
                        ),
                    )

        # How many h-rows fit a 512-column matmul block
        rb = max(1, min(512 // w, h))

        for g, (b, h0, hr) in enumerate(groups):
            xb = xbs[b]
            xg = xb[:].rearrange("c (j h w) -> c j h w", j=n_ch, h=h)[
                :, :, h0 : h0 + hr
            ]

            # hx broadcast to all partitions: psum[m, (i,k)] = sum_c
            # wh[c] x[c, h0+i, k], accumulated over the two c-chunks,
            # one matmul pair per rb-row (512-col) block.
            pt = psp.tile([P, hr * w], f32)
            for q in range(0, hr, rb):
                qn = min(rb, hr - q)
                reg = pt[:, q * w : (q + qn) * w]
                for ch in range(n_ch):
                    nc.tensor.matmul(
                        reg,
                        lhsT=wh_bcast[:, ch * P : (ch + 1) * P],
                        rhs=xg[:, ch, q : q + qn].rearrange("c h w -> c (h w)"),
                        start=(ch == 0),
                        stop=(ch == n_ch - 1),
                    )

            # s = row-sum(hx) + W*bh : full-lane PSUM reduce, tiny bias add
            rsg = rpool.tile([P, hr], f32)
            nc.vector.reduce_sum(
                rsg[:], pt[:].rearrange("p (h w) -> p h w", w=w), axis=X
            )
            s128 = spool.tile([P, hr], f32)
            nc.scalar.add(s128[:], rsg[:], biasW[:])

            # o = s * x quantized to fp16, then store this group's rows
            ot = opool.tile([P, n_ch * hr * w], f16)
            eng = nc.gpsimd if g in pool_mult else nc.vector
            eng.tensor_mul(
                ot[:].rearrange("c (j h w) -> c j h w", j=n_ch, h=hr),
                xg,
                s128[:, None, :, None].broadcast_to((P, n_ch, hr, w)),
            )
            nc.scalar.dma_start(
                o[b, :, h0 : h0 + hr].rearrange("(j c) h w -> c j h w", c=P),
                ot[:].rearrange("c (j h w) -> c j h w", j=n_ch, h=hr),
            )
    _split_multi_waits(nc, mybir)
    return nc


def get_nc(bs=BS, c=C, h=H, w=W):
    key = (bs, c, h, w)
    if key not in _CACHE:
        _CACHE[key] = _build(bs, c, h, w)
    return _CACHE[key]


def kernel(x, wf, bf, wg, bg, wh, bh, **_unused):
    from concourse.bass_utils import run_bass_kernel_spmd

    x = np.ascontiguousarray(np.asarray(x, dtype=np.float32))
    wh = np.ascontiguousarray(np.asarray(wh, dtype=np.float32))
    bh = np.ascontiguousarray(np.asarray(bh, dtype=np.float32))

    in_maps = [
        {"x": x[k * BS : (k + 1) * BS], "wh": wh, "bh": bh} for k in range(N_CORES)
    ]
    # Tile scheduling is nondeterministic build-to-build and a rare schedule
    # can deadlock on hardware (NRT unrecoverable). Rebuilding produces a
    # fresh schedule, so retry with a clean build on any execution failure.
    last_err = None
    for attempt in range(3):
        try:
            nc = get_nc()
            res = run_bass_kernel_spmd(nc, in_maps, core_ids=list(range(N_CORES)))
            return np.concatenate(
                [
                    np.asarray(res.results[k]["o"], dtype=np.float32)
                    for k in range(N_CORES)
                ],
                axis=0,
            )
        except Exception as e:  # rebuild with a new schedule and retry
            last_err = e
            _CACHE.clear()
    raise last_err
